# revision 2
# baseline (speedup 1.0000x reference)
"""BoundaryDiceLoss Trainium2 kernel, v5.

Structure (per image, pixels as [128, FW=2048] bf16 tiles):
- ACT: exp x5 (one inst), ln(s), r2 = exp(-ln_s - ln2) = 1/(2s).
- DVE: boundary map w, softmax denom s (4 adds), r2w = r2*w,
  masks m_c (tensor_scalar 4x mode, accum_out -> N counts for free),
  Em_c = E_c*m_c (5 TT).
- PE: every needed sum is a pair-dot  sum_pix A*B  computed as 16
  chunked [128,128] gram matmuls accumulating in PSUM; only the
  gram diagonals carry signal, extracted on HOST from the evacuated
  gram tiles (bf16).

Pair-dots per image (21 gram blocks of 128 cols each, PSUM layout):
  PA @0    : lhsT=r2 , rhs=E_0..E_3   -> S1h[0..3]
  PB @512  : lhsT=r2 , rhs=Em_0..Em_3 -> S2h[0..3]
  PD @1024 : lhsT=r2w, rhs=E_1..E_4   -> S4h[1..4]
  PE @1536 : lhsT=r2w, rhs=Em_1..Em_4 -> S5h[1..4]
  PF @2048 : lhsT=w  , rhs=m_1..m_4   -> NW[1..4]
  PC @2560 : lhsT=r2 , rhs=Em_4       -> S2h[4]
Region-major matmul order so each PSUM region evacuates (ACT copy,
f32->bf16) independently -> no inter-image PSUM bubble.
Host finalize: S1[4] and S4[0] derived from sum constraints.
"""
import sys

sys.path.insert(0, "/opt/trn_rl_repo")

import numpy as np

NUM_CLASSES = 5
BOUNDARY_WEIGHT = 0.8
EPS = 1e-6
N_CORES = 8
GW = 2688  # gram output width: 21 blocks of 128

_CACHE = {}

ACT_SET = "natural_log_exp_and_others"

GP_OFFLOAD = False  # GPSIMD TensorTensor fails walrus codegen ("engine check")
WIDE_MM = False     # rank-4 moving APs fail the walrus ISA check


def _build(BL, C, H, W, repeat=1, fake_inputs=False):
    import types
    import contextlib
    import concourse.bacc as bacc
    import concourse.tile as tile
    import concourse.mybir as mybir
    import concourse.bass as bass
    import bass_rust as _bass_rust
    from concourse.hw_specs import get_activation_tables

    AF = mybir.ActivationFunctionType
    OP = mybir.AluOpType
    f32 = mybir.dt.float32
    bf16 = mybir.dt.bfloat16

    S = H // 128          # 4
    FW = S * W            # 2048
    HB = W + 2
    NCH = FW // 128       # 16 gram chunks

    nc = bacc.Bacc("TRN2", target_bir_lowering=False, debug=False)

    def _single_set_table_loads(self):
        has_activation = any(
            isinstance(i, mybir.InstActivation)
            for b in self.main_func.blocks
            for i in b.instructions
        )
        if not has_activation:
            return
        tables = get_activation_tables(self.m.arch)
        only = {
            name: (funcs if name == ACT_SET else set())
            for name, funcs in tables.items()
        }
        _bass_rust.insert_act_table_loads(self, list(only.items()))

    nc.insert_act_table_loads = types.MethodType(_single_set_table_loads, nc)

    LN2 = 0.6931471805599453
    _t = nc.alloc_sbuf_tensor("const-float32-r2bias", [128, 1], f32)
    nc.vector.memset(_t.ap(), -LN2)
    nc.const_aps.aps[(f32, -LN2)] = _t.ap()
    nc.all_engine_barrier()

    ikind = "Internal" if fake_inputs else "ExternalInput"
    pred_d = nc.dram_tensor("pred", [BL, C, H, W], bf16, kind=ikind).ap()
    # target padded with an edge-duplicated row above and below each image
    targ_d = nc.dram_tensor("target", [BL, H + 2, W], bf16, kind=ikind).ap()
    g_d = nc.dram_tensor("gsums", [BL, 128, GW], bf16, kind="ExternalOutput").ap()
    n_d = nc.dram_tensor("nsums", [128, 5 * BL], f32, kind="ExternalOutput").ap()

    with tile.TileContext(nc) as tc:
        with (
            tc.tile_pool(name="px", bufs=2) as px,
            tc.tile_pool(name="pt", bufs=1) as pt,
            tc.tile_pool(name="ps", bufs=1) as ps,
            tc.tile_pool(name="pr", bufs=2) as pr,
            tc.tile_pool(name="pm", bufs=2) as pm,
            tc.tile_pool(name="pg", bufs=2) as pg,
            tc.tile_pool(name="pn", bufs=1) as pn,
            tc.tile_pool(name="pps", bufs=1, space=bass.MemorySpace.PSUM) as pps,
        ):
            # one psum tile per gram region so a region's matmuls never
            # wait on another region's evacuation
            psA = pps.tile([128, 512], f32, tag="psA", name="psA")
            psB = pps.tile([128, 512], f32, tag="psB", name="psB")
            psD = pps.tile([128, 512], f32, tag="psD", name="psD")
            psE = pps.tile([128, 512], f32, tag="psE", name="psE")
            psF = pps.tile([128, 512], f32, tag="psF", name="psF")
            psC = pps.tile([128, 128], f32, tag="psC", name="psC")
            ncol = pn.tile([128, 5 * BL], f32, tag="ncol", name="ncol")

            def p1_dma_exp(b):
                """Input DMAs (SP queue) + exp on ACT."""
                xg = px.tile([128, C, S, W], bf16, tag="xg")
                nc.sync.dma_start(
                    xg[:], pred_d[b].rearrange("c (s p) w -> p c s w", p=128)
                )
                t8 = pt.tile([128, S, W], bf16, tag="t")
                nc.sync.dma_start(
                    t8[:], targ_d[b][1 : H + 1].rearrange("(s p) w -> p s w", p=128)
                )
                tu8 = pt.tile([128, S, W], bf16, tag="tu")
                nc.sync.dma_start(
                    tu8[:], targ_d[b][0:H].rearrange("(s p) w -> p s w", p=128)
                )
                td8 = pt.tile([128, S, W], bf16, tag="td")
                nc.sync.dma_start(
                    td8[:], targ_d[b][2 : H + 2].rearrange("(s p) w -> p s w", p=128)
                )
                # per-class exp so the s-add chain can start after E0,E1
                Ec = xg[:].rearrange("p c s w -> p c (s w)")
                for c in range(C):
                    nc.scalar.activation(Ec[:, c], Ec[:, c], AF.Exp)
                return {"b": b, "t8": t8, "tu8": tu8, "td8": td8, "Ec": Ec}

            def p1_early_dve(tl):
                """Masks + boundary map: DVE work with no exp dependency."""
                b, t8 = tl["b"], tl["t8"]
                tf = t8[:].rearrange("p s w -> p (s w)")
                m = pm.tile([128, C, FW], bf16, tag="m")
                for c in range(C):
                    nc.vector.tensor_scalar(
                        m[:, c], tf[:], float(c), 0.0, op0=OP.is_equal,
                        op1=OP.add,
                        accum_out=ncol[:, b * 5 + c : b * 5 + c + 1],
                    )
                vn1 = ps.tile([128, FW], bf16, tag="scrA")
                nc.vector.tensor_tensor(
                    vn1[:].rearrange("p (s w) -> p s w", s=S), t8[:], tl["tu8"][:],
                    op=OP.not_equal,
                )
                vn2 = ps.tile([128, S, W], bf16, tag="scrB")
                nc.vector.tensor_tensor(vn2[:], t8[:], tl["td8"][:], op=OP.not_equal)
                hbuf = ps.tile([128, S, HB], bf16, tag="hbuf")
                nc.vector.memset(hbuf[:, :, 0:1], 0.0)
                nc.vector.memset(hbuf[:, :, W : W + 1], 0.0)
                nc.vector.tensor_tensor(
                    hbuf[:, :, 1:W], t8[:, :, 0 : W - 1], t8[:, :, 1:W],
                    op=OP.not_equal,
                )
                nc.vector.tensor_tensor(
                    vn1[:], vn1[:], vn2[:].rearrange("p s w -> p (s w)"),
                    op=OP.max,
                )
                nc.vector.tensor_tensor(
                    vn2[:], hbuf[:, :, 0:W], hbuf[:, :, 1 : W + 1], op=OP.max,
                )
                nc.vector.tensor_tensor(
                    vn1[:], vn1[:], vn2[:].rearrange("p s w -> p (s w)"),
                    op=OP.max,
                )
                # w = boundary AND (t>0)  ==  c1 > m_0
                w = pr.tile([128, FW], bf16, tag="w")
                nc.vector.tensor_tensor(w[:], vn1[:], m[:, 0], op=OP.is_gt)
                tl["m"] = m
                tl["w"] = w

            def p1_s(tl):
                """s (DVE, needs exp) -> ln/r2 (ACT)."""
                Ec = tl["Ec"]
                scrA = ps.tile([128, FW], bf16, tag="scrA")
                nc.vector.tensor_tensor(scrA[:], Ec[:, 0], Ec[:, 1], op=OP.add)
                scrB = ps.tile([128, FW], bf16, tag="scrB")
                nc.vector.tensor_tensor(
                    scrB[:], Ec[:, 2], Ec[:, 3], op=OP.add)
                nc.vector.tensor_tensor(scrA[:], scrA[:], scrB[:], op=OP.add)
                sden = ps.tile([128, FW], bf16, tag="sden")
                nc.vector.tensor_tensor(sden[:], scrA[:], Ec[:, 4], op=OP.add)
                ln_s = ps.tile([128, FW], f32, tag="lns")
                nc.scalar.activation(ln_s[:], sden[:], AF.Ln)
                r2 = pr.tile([128, FW], bf16, tag="r2")
                nc.scalar.activation(r2[:], ln_s[:], AF.Exp, scale=-1.0, bias=-LN2)
                tl["r2"] = r2

            def p1_em(tl):
                """r2w then masked exponentials (DVE)."""
                r2w = pr.tile([128, FW], bf16, tag="r2w")
                nc.vector.tensor_tensor(r2w[:], tl["r2"][:], tl["w"][:], op=OP.mult)
                tl["r2w"] = r2w
                Ec = tl["Ec"]
                Em = pm.tile([128, C, FW], bf16, tag="Em")
                for c in range(C):
                    nc.vector.tensor_tensor(Em[:, c], Ec[:, c], tl["m"][:, c], op=OP.mult)
                tl["Em"] = Em

            def phase1(b):
                # warmup order: softmax chain first so PE can start ASAP
                tl = p1_dma_exp(b)
                p1_s(tl)
                p1_early_dve(tl)
                p1_em(tl)
                return tl

            def mm_region(pt_, statn, mov, nb):
                for k in range(NCH):
                    kk = slice(128 * k, 128 * (k + 1))
                    nc.tensor.matmul(
                        pt_[:, 0 : nb * 128],
                        statn[:, kk],
                        mov[..., kk],
                        start=(k == 0), stop=(k == NCH - 1),
                        skip_group_check=True,
                    )

            def evac(gsb, pt_, off, nb):
                nc.scalar.activation(
                    gsb[:, off : off + nb * 128], pt_[:, 0 : nb * 128], AF.Copy,
                )

            def phase2a(tl, gsb):
                """Gram regions not needing Em: PA, PF, PD (+ evacs)."""
                mm_region(psA, tl["r2"], tl["Ec"][:, 0:4], 4)
                evac(gsb, psA, 0, 4)
                mm_region(psF, tl["w"], tl["m"][:, 1:5], 4)
                evac(gsb, psF, 2048, 4)
                mm_region(psD, tl["r2w"], tl["Ec"][:, 1:5], 4)
                evac(gsb, psD, 1024, 4)

            def phase2b(tl, gsb):
                """Em gram regions: PB, PE2, PC (+ evacs) and result DMA."""
                mm_region(psB, tl["r2"], tl["Em"][:, 0:4], 4)
                evac(gsb, psB, 512, 4)
                mm_region(psE, tl["r2w"], tl["Em"][:, 1:5], 4)
                evac(gsb, psE, 1536, 4)
                mm_region(psC, tl["r2"], tl["Em"][:, 4:5], 1)
                evac(gsb, psC, 2560, 1)
                nc.scalar.dma_start(g_d[tl["b"]], gsb[:])

            # Software-pipelined schedule. Per slot b (steady state):
            #   PE: regions without Em for b, then Em regions for b
            #   ACT: evacs(b) interleave; exp(b+1) issued between the two
            #        PE groups so it runs during PE(b)'s Em regions
            #   DVE: masks/wmap(b+1) during exp(b+1), then s/Em/r2w(b+1)
            loop_cm = tc.For_i(0, repeat) if repeat > 1 else contextlib.nullcontext()
            with loop_cm:
                tl = phase1(0)
                for b in range(BL):
                    gsb = pg.tile([128, GW], bf16, tag="gsb")
                    phase2a(tl, gsb)
                    nxt = None
                    if b + 1 < BL:
                        nxt = p1_dma_exp(b + 1)
                        p1_early_dve(nxt)
                    phase2b(tl, gsb)
                    if nxt is not None:
                        p1_s(nxt)
                        p1_em(nxt)
                        tl = nxt

            nc.sync.dma_start(n_d[:], ncol[:])

    nc.compile()
    return nc


def _get_nc(BL, C, H, W, repeat=1, **kw):
    key = (BL, C, H, W, repeat, tuple(sorted(kw.items())))
    if key not in _CACHE:
        _CACHE[key] = _build(BL, C, H, W, repeat, **kw)
    return _CACHE[key]


def _finalize(results, BL, C, npix=512 * 512):
    dice_std_all = []
    dice_b_all = []
    for res in results:
        g = np.asarray(res["gsums"]).astype(np.float64)   # [BL, 128, GW]
        ns = np.asarray(res["nsums"]).astype(np.float64)  # [128, 5*BL]
        q = np.arange(128)
        for b in range(BL):
            blocks = g[b].reshape(128, GW // 128, 128)    # [q, block, col]
            d = blocks[q, :, q]                           # [128, 21]
            bs = d.sum(axis=0)                            # 21 block sums
            S1h = bs[0:4]
            S2h = np.concatenate([bs[4:8], bs[20:21]])
            S4h = bs[8:12]
            S5h = bs[12:16]
            NWh = bs[16:20]
            N = ns[:, b * 5 : (b + 1) * 5].sum(axis=0)

            S1 = np.zeros(5)
            S1[:4] = 2.0 * S1h
            S1[4] = npix - S1[:4].sum()
            S2 = 2.0 * S2h
            NW = np.zeros(5)
            NW[1:] = NWh
            wtot = NWh.sum()
            S4 = np.zeros(5)
            S4[1:] = 2.0 * S4h
            S4[0] = wtot - S4[1:].sum()
            S5 = np.zeros(5)
            S5[1:] = 2.0 * S5h

            dice_std_all.append((2.0 * S2 + EPS) / (S1 + N + EPS))
            dice_b_all.append((2.0 * S5 + EPS) / (S4 + NW + EPS))
    loss_std = 1.0 - np.stack(dice_std_all).mean()
    loss_b = 1.0 - np.stack(dice_b_all).mean()
    return np.float32(
        (1.0 - BOUNDARY_WEIGHT) * loss_std + BOUNDARY_WEIGHT * loss_b
    )


def kernel(pred, target):
    import ml_dtypes
    from concourse.bass_utils import run_bass_kernel_spmd

    pred = np.ascontiguousarray(
        np.asarray(pred, dtype=np.float32).astype(ml_dtypes.bfloat16)
    )
    t = np.asarray(target).astype(ml_dtypes.bfloat16)
    target = np.ascontiguousarray(
        np.concatenate([t[:, 0:1], t, t[:, -1:]], axis=1)
    )
    B, C, H, W = pred.shape
    assert B % N_CORES == 0
    BL = B // N_CORES

    nc = _get_nc(BL, C, H, W)
    in_maps = [
        {
            "pred": pred[i * BL : (i + 1) * BL],
            "target": target[i * BL : (i + 1) * BL],
        }
        for i in range(N_CORES)
    ]
    res = run_bass_kernel_spmd(nc, in_maps, list(range(N_CORES)))
    return _finalize([res.results[i] for i in range(N_CORES)], BL, C)


# revision 3
# speedup vs baseline: 1.0259x; 1.0259x over previous
"""BoundaryDiceLoss Trainium2 kernel, v5.

Structure (per image, pixels as [128, FW=2048] bf16 tiles):
- ACT: exp x5 (one inst), ln(s), r2 = exp(-ln_s - ln2) = 1/(2s).
- DVE: boundary map w, softmax denom s (4 adds), r2w = r2*w,
  masks m_c (tensor_scalar 4x mode, accum_out -> N counts for free),
  Em_c = E_c*m_c (5 TT).
- PE: every needed sum is a pair-dot  sum_pix A*B  computed as 16
  chunked [128,128] gram matmuls accumulating in PSUM; only the
  gram diagonals carry signal, extracted on HOST from the evacuated
  gram tiles (bf16).

Pair-dots per image (21 gram blocks of 128 cols each, PSUM layout):
  PA @0    : lhsT=r2 , rhs=E_0..E_3   -> S1h[0..3]
  PB @512  : lhsT=r2 , rhs=Em_0..Em_3 -> S2h[0..3]
  PD @1024 : lhsT=r2w, rhs=E_1..E_4   -> S4h[1..4]
  PE @1536 : lhsT=r2w, rhs=Em_1..Em_4 -> S5h[1..4]
  PF @2048 : lhsT=w  , rhs=m_1..m_4   -> NW[1..4]
  PC @2560 : lhsT=r2 , rhs=Em_4       -> S2h[4]
Region-major matmul order so each PSUM region evacuates (ACT copy,
f32->bf16) independently -> no inter-image PSUM bubble.
Host finalize: S1[4] and S4[0] derived from sum constraints.
"""
import sys

sys.path.insert(0, "/opt/trn_rl_repo")

import numpy as np

NUM_CLASSES = 5
BOUNDARY_WEIGHT = 0.8
EPS = 1e-6
N_CORES = 8
GW = 2688  # gram output width: 21 blocks of 128

_CACHE = {}

ACT_SET = "natural_log_exp_and_others"

GP_OFFLOAD = False  # GPSIMD TensorTensor fails walrus codegen ("engine check")
WIDE_MM = False     # rank-4 moving APs fail the walrus ISA check


def _build(BL, C, H, W, repeat=1, fake_inputs=False):
    import types
    import contextlib
    import concourse.bacc as bacc
    import concourse.tile as tile
    import concourse.mybir as mybir
    import concourse.bass as bass
    import bass_rust as _bass_rust
    from concourse.hw_specs import get_activation_tables

    AF = mybir.ActivationFunctionType
    OP = mybir.AluOpType
    f32 = mybir.dt.float32
    bf16 = mybir.dt.bfloat16

    S = H // 128          # 4
    FW = S * W            # 2048
    HB = W + 2
    NCH = FW // 128       # 16 gram chunks

    nc = bacc.Bacc("TRN2", target_bir_lowering=False, debug=False)

    def _single_set_table_loads(self):
        has_activation = any(
            isinstance(i, mybir.InstActivation)
            for b in self.main_func.blocks
            for i in b.instructions
        )
        if not has_activation:
            return
        tables = get_activation_tables(self.m.arch)
        only = {
            name: (funcs if name == ACT_SET else set())
            for name, funcs in tables.items()
        }
        _bass_rust.insert_act_table_loads(self, list(only.items()))

    nc.insert_act_table_loads = types.MethodType(_single_set_table_loads, nc)

    LN2 = 0.6931471805599453
    _t = nc.alloc_sbuf_tensor("const-float32-r2bias", [128, 1], f32)
    nc.vector.memset(_t.ap(), -LN2)
    nc.const_aps.aps[(f32, -LN2)] = _t.ap()
    nc.all_engine_barrier()

    ikind = "Internal" if fake_inputs else "ExternalInput"
    pred_d = nc.dram_tensor("pred", [BL, C, H, W], bf16, kind=ikind).ap()
    # target padded with an edge-duplicated row above and below each image
    targ_d = nc.dram_tensor("target", [BL, H + 2, W], bf16, kind=ikind).ap()
    g_d = nc.dram_tensor("gsums", [BL, 128, GW], bf16, kind="ExternalOutput").ap()
    n_d = nc.dram_tensor("nsums", [128, 5 * BL], f32, kind="ExternalOutput").ap()

    with tile.TileContext(nc) as tc:
        with (
            tc.tile_pool(name="px", bufs=2) as px,
            tc.tile_pool(name="pt", bufs=1) as pt,
            tc.tile_pool(name="ps", bufs=1) as ps,
            tc.tile_pool(name="pr", bufs=2) as pr,
            tc.tile_pool(name="pm", bufs=2) as pm,
            tc.tile_pool(name="pg", bufs=2) as pg,
            tc.tile_pool(name="pn", bufs=1) as pn,
            tc.tile_pool(name="pps", bufs=1, space=bass.MemorySpace.PSUM) as pps,
        ):
            # one psum tile per gram region so a region's matmuls never
            # wait on another region's evacuation
            psA = pps.tile([128, 512], f32, tag="psA", name="psA")
            psB = pps.tile([128, 512], f32, tag="psB", name="psB")
            psD = pps.tile([128, 512], f32, tag="psD", name="psD")
            psE = pps.tile([128, 512], f32, tag="psE", name="psE")
            psF = pps.tile([128, 512], f32, tag="psF", name="psF")
            psC = pps.tile([128, 128], f32, tag="psC", name="psC")
            ncol = pn.tile([128, 5 * BL], f32, tag="ncol", name="ncol")

            def p1_dma_exp(b):
                """Input DMAs (SP queue) + exp on ACT."""
                xg = px.tile([128, C, S, W], bf16, tag="xg")
                Ecv = xg[:].rearrange("p c s w -> p c (s w)")
                for c in range(C):
                    nc.sync.dma_start(
                        xg[:, c], pred_d[b][c].rearrange("(s p) w -> p s w", p=128)
                    )
                    nc.scalar.activation(Ecv[:, c], Ecv[:, c], AF.Exp)
                t8 = pt.tile([128, S, W], bf16, tag="t")
                nc.sync.dma_start(
                    t8[:], targ_d[b][1 : H + 1].rearrange("(s p) w -> p s w", p=128)
                )
                tu8 = pt.tile([128, S, W], bf16, tag="tu")
                nc.sync.dma_start(
                    tu8[:], targ_d[b][0:H].rearrange("(s p) w -> p s w", p=128)
                )
                td8 = pt.tile([128, S, W], bf16, tag="td")
                nc.sync.dma_start(
                    td8[:], targ_d[b][2 : H + 2].rearrange("(s p) w -> p s w", p=128)
                )
                return {"b": b, "t8": t8, "tu8": tu8, "td8": td8, "Ec": Ecv}

            def p1_early_dve(tl):
                """Masks + boundary map: DVE work with no exp dependency."""
                b, t8 = tl["b"], tl["t8"]
                tf = t8[:].rearrange("p s w -> p (s w)")
                m = pm.tile([128, C, FW], bf16, tag="m")
                for c in range(C):
                    nc.vector.tensor_scalar(
                        m[:, c], tf[:], float(c), 0.0, op0=OP.is_equal,
                        op1=OP.add,
                        accum_out=ncol[:, b * 5 + c : b * 5 + c + 1],
                    )
                vn1 = ps.tile([128, FW], bf16, tag="scrA")
                nc.vector.tensor_tensor(
                    vn1[:].rearrange("p (s w) -> p s w", s=S), t8[:], tl["tu8"][:],
                    op=OP.not_equal,
                )
                vn2 = ps.tile([128, S, W], bf16, tag="scrB")
                nc.vector.tensor_tensor(vn2[:], t8[:], tl["td8"][:], op=OP.not_equal)
                hbuf = ps.tile([128, S, HB], bf16, tag="hbuf")
                nc.vector.memset(hbuf[:, :, 0:1], 0.0)
                nc.vector.memset(hbuf[:, :, W : W + 1], 0.0)
                nc.vector.tensor_tensor(
                    hbuf[:, :, 1:W], t8[:, :, 0 : W - 1], t8[:, :, 1:W],
                    op=OP.not_equal,
                )
                nc.vector.tensor_tensor(
                    vn1[:], vn1[:], vn2[:].rearrange("p s w -> p (s w)"),
                    op=OP.max,
                )
                nc.vector.tensor_tensor(
                    vn2[:], hbuf[:, :, 0:W], hbuf[:, :, 1 : W + 1], op=OP.max,
                )
                nc.vector.tensor_tensor(
                    vn1[:], vn1[:], vn2[:].rearrange("p s w -> p (s w)"),
                    op=OP.max,
                )
                # w = boundary AND (t>0)  ==  c1 > m_0
                w = pr.tile([128, FW], bf16, tag="w")
                nc.vector.tensor_tensor(w[:], vn1[:], m[:, 0], op=OP.is_gt)
                tl["m"] = m
                tl["w"] = w

            def p1_s(tl):
                """s (DVE, needs exp) -> ln/r2 (ACT)."""
                Ec = tl["Ec"]
                scrA = ps.tile([128, FW], bf16, tag="scrA")
                nc.vector.tensor_tensor(scrA[:], Ec[:, 0], Ec[:, 1], op=OP.add)
                scrB = ps.tile([128, FW], bf16, tag="scrB")
                nc.vector.tensor_tensor(
                    scrB[:], Ec[:, 2], Ec[:, 3], op=OP.add)
                nc.vector.tensor_tensor(scrA[:], scrA[:], scrB[:], op=OP.add)
                sden = ps.tile([128, FW], bf16, tag="sden")
                nc.vector.tensor_tensor(sden[:], scrA[:], Ec[:, 4], op=OP.add)
                ln_s = ps.tile([128, FW], f32, tag="lns")
                nc.scalar.activation(ln_s[:], sden[:], AF.Ln)
                r2 = pr.tile([128, FW], bf16, tag="r2")
                nc.scalar.activation(r2[:], ln_s[:], AF.Exp, scale=-1.0, bias=-LN2)
                tl["r2"] = r2

            def p1_em(tl):
                """r2w then masked exponentials (DVE)."""
                r2w = pr.tile([128, FW], bf16, tag="r2w")
                nc.vector.tensor_tensor(r2w[:], tl["r2"][:], tl["w"][:], op=OP.mult)
                tl["r2w"] = r2w
                Ec = tl["Ec"]
                Em = pm.tile([128, C, FW], bf16, tag="Em")
                for c in range(C):
                    nc.vector.tensor_tensor(
                        Em[:, c], Ec[:, c], tl["m"][:, c], op=OP.mult)
                tl["Em"] = Em

            def phase1(b):
                # warmup order: softmax chain first so PE can start ASAP
                tl = p1_dma_exp(b)
                p1_s(tl)
                p1_early_dve(tl)
                p1_em(tl)
                return tl

            def mm_region(pt_, statn, mov, nb):
                for k in range(NCH):
                    kk = slice(128 * k, 128 * (k + 1))
                    nc.tensor.matmul(
                        pt_[:, 0 : nb * 128],
                        statn[:, kk],
                        mov[..., kk],
                        start=(k == 0), stop=(k == NCH - 1),
                        skip_group_check=True,
                    )

            def evac(gsb, pt_, off, nb):
                nc.scalar.activation(
                    gsb[:, off : off + nb * 128], pt_[:, 0 : nb * 128], AF.Copy,
                )

            def phase2a(tl, gsb):
                """Gram regions not needing Em: PA, PF, PD (+ evacs)."""
                mm_region(psA, tl["r2"], tl["Ec"][:, 0:4], 4)
                evac(gsb, psA, 0, 4)
                mm_region(psF, tl["w"], tl["m"][:, 1:5], 4)
                evac(gsb, psF, 2048, 4)
                mm_region(psD, tl["r2w"], tl["Ec"][:, 1:5], 4)
                evac(gsb, psD, 1024, 4)

            def phase2b(tl, gsb):
                """Em gram regions: PB, PE2, PC (+ evacs) and result DMA."""
                mm_region(psB, tl["r2"], tl["Em"][:, 0:4], 4)
                evac(gsb, psB, 512, 4)
                mm_region(psE, tl["r2w"], tl["Em"][:, 1:5], 4)
                evac(gsb, psE, 1536, 4)
                mm_region(psC, tl["r2"], tl["Em"][:, 4:5], 1)
                evac(gsb, psC, 2560, 1)
                nc.scalar.dma_start(g_d[tl["b"]], gsb[:])

            # Software-pipelined schedule. Per slot b (steady state):
            #   PE: regions without Em for b, then Em regions for b
            #   ACT: evacs(b) interleave; exp(b+1) issued between the two
            #        PE groups so it runs during PE(b)'s Em regions
            #   DVE: masks/wmap(b+1) during exp(b+1), then s/Em/r2w(b+1)
            loop_cm = tc.For_i(0, repeat) if repeat > 1 else contextlib.nullcontext()
            with loop_cm:
                tl = phase1(0)
                for b in range(BL):
                    gsb = pg.tile([128, GW], bf16, tag="gsb")
                    phase2a(tl, gsb)
                    nxt = None
                    if b + 1 < BL:
                        nxt = p1_dma_exp(b + 1)
                        p1_early_dve(nxt)
                    phase2b(tl, gsb)
                    if nxt is not None:
                        p1_s(nxt)
                        p1_em(nxt)
                        tl = nxt

            nc.sync.dma_start(n_d[:], ncol[:])

    nc.compile()
    return nc


def _get_nc(BL, C, H, W, repeat=1, **kw):
    key = (BL, C, H, W, repeat, tuple(sorted(kw.items())))
    if key not in _CACHE:
        _CACHE[key] = _build(BL, C, H, W, repeat, **kw)
    return _CACHE[key]


def _finalize(results, BL, C, npix=512 * 512):
    dice_std_all = []
    dice_b_all = []
    for res in results:
        g = np.asarray(res["gsums"]).astype(np.float64)   # [BL, 128, GW]
        ns = np.asarray(res["nsums"]).astype(np.float64)  # [128, 5*BL]
        q = np.arange(128)
        for b in range(BL):
            blocks = g[b].reshape(128, GW // 128, 128)    # [q, block, col]
            d = blocks[q, :, q]                           # [128, 21]
            bs = d.sum(axis=0)                            # 21 block sums
            S1h = bs[0:4]
            S2h = np.concatenate([bs[4:8], bs[20:21]])
            S4h = bs[8:12]
            S5h = bs[12:16]
            NWh = bs[16:20]
            N = ns[:, b * 5 : (b + 1) * 5].sum(axis=0)

            S1 = np.zeros(5)
            S1[:4] = 2.0 * S1h
            S1[4] = npix - S1[:4].sum()
            S2 = 2.0 * S2h
            NW = np.zeros(5)
            NW[1:] = NWh
            wtot = NWh.sum()
            S4 = np.zeros(5)
            S4[1:] = 2.0 * S4h
            S4[0] = wtot - S4[1:].sum()
            S5 = np.zeros(5)
            S5[1:] = 2.0 * S5h

            dice_std_all.append((2.0 * S2 + EPS) / (S1 + N + EPS))
            dice_b_all.append((2.0 * S5 + EPS) / (S4 + NW + EPS))
    loss_std = 1.0 - np.stack(dice_std_all).mean()
    loss_b = 1.0 - np.stack(dice_b_all).mean()
    return np.float32(
        (1.0 - BOUNDARY_WEIGHT) * loss_std + BOUNDARY_WEIGHT * loss_b
    )


def kernel(pred, target):
    import ml_dtypes
    from concourse.bass_utils import run_bass_kernel_spmd

    pred = np.ascontiguousarray(
        np.asarray(pred, dtype=np.float32).astype(ml_dtypes.bfloat16)
    )
    t = np.asarray(target).astype(ml_dtypes.bfloat16)
    target = np.ascontiguousarray(
        np.concatenate([t[:, 0:1], t, t[:, -1:]], axis=1)
    )
    B, C, H, W = pred.shape
    assert B % N_CORES == 0
    BL = B // N_CORES

    nc = _get_nc(BL, C, H, W)
    in_maps = [
        {
            "pred": pred[i * BL : (i + 1) * BL],
            "target": target[i * BL : (i + 1) * BL],
        }
        for i in range(N_CORES)
    ]
    res = run_bass_kernel_spmd(nc, in_maps, list(range(N_CORES)))
    return _finalize([res.results[i] for i in range(N_CORES)], BL, C)


# revision 4
# speedup vs baseline: 1.0273x; 1.0013x over previous
"""BoundaryDiceLoss Trainium2 kernel, v5.

Structure (per image, pixels as [128, FW=2048] bf16 tiles):
- ACT: exp x5 (one inst), ln(s), r2 = exp(-ln_s - ln2) = 1/(2s).
- DVE: boundary map w, softmax denom s (4 adds), r2w = r2*w,
  masks m_c (tensor_scalar 4x mode, accum_out -> N counts for free),
  Em_c = E_c*m_c (5 TT).
- PE: every needed sum is a pair-dot  sum_pix A*B  computed as 16
  chunked [128,128] gram matmuls accumulating in PSUM; only the
  gram diagonals carry signal, extracted on HOST from the evacuated
  gram tiles (bf16).

Pair-dots per image (21 gram blocks of 128 cols each, PSUM layout):
  PA @0    : lhsT=r2 , rhs=E_0..E_3   -> S1h[0..3]
  PB @512  : lhsT=r2 , rhs=Em_0..Em_3 -> S2h[0..3]
  PD @1024 : lhsT=r2w, rhs=E_1..E_4   -> S4h[1..4]
  PE @1536 : lhsT=r2w, rhs=Em_1..Em_4 -> S5h[1..4]
  PF @2048 : lhsT=w  , rhs=m_1..m_4   -> NW[1..4]
  PC @2560 : lhsT=r2 , rhs=Em_4       -> S2h[4]
Region-major matmul order so each PSUM region evacuates (ACT copy,
f32->bf16) independently -> no inter-image PSUM bubble.
Host finalize: S1[4] and S4[0] derived from sum constraints.
"""
import sys

sys.path.insert(0, "/opt/trn_rl_repo")

import numpy as np

NUM_CLASSES = 5
BOUNDARY_WEIGHT = 0.8
EPS = 1e-6
N_CORES = 8
GW = 2688  # gram output width: 21 blocks of 128

_CACHE = {}

ACT_SET = "natural_log_exp_and_others"

GP_OFFLOAD = False  # GPSIMD TensorTensor fails walrus codegen ("engine check")
WIDE_MM = False     # rank-4 moving APs fail the walrus ISA check


def _build(BL, C, H, W, repeat=1, fake_inputs=False):
    import types
    import contextlib
    import concourse.bacc as bacc
    import concourse.tile as tile
    import concourse.mybir as mybir
    import concourse.bass as bass
    import bass_rust as _bass_rust
    from concourse.hw_specs import get_activation_tables

    AF = mybir.ActivationFunctionType
    OP = mybir.AluOpType
    f32 = mybir.dt.float32
    bf16 = mybir.dt.bfloat16

    S = H // 128          # 4
    FW = S * W            # 2048
    HB = W + 2
    NCH = FW // 128       # 16 gram chunks

    nc = bacc.Bacc("TRN2", target_bir_lowering=False, debug=False)

    def _single_set_table_loads(self):
        has_activation = any(
            isinstance(i, mybir.InstActivation)
            for b in self.main_func.blocks
            for i in b.instructions
        )
        if not has_activation:
            return
        tables = get_activation_tables(self.m.arch)
        only = {
            name: (funcs if name == ACT_SET else set())
            for name, funcs in tables.items()
        }
        _bass_rust.insert_act_table_loads(self, list(only.items()))

    nc.insert_act_table_loads = types.MethodType(_single_set_table_loads, nc)

    LN2 = 0.6931471805599453
    _t = nc.alloc_sbuf_tensor("const-float32-r2bias", [128, 1], f32)
    nc.vector.memset(_t.ap(), -LN2)
    nc.const_aps.aps[(f32, -LN2)] = _t.ap()
    nc.all_engine_barrier()

    ikind = "Internal" if fake_inputs else "ExternalInput"
    pred_d = nc.dram_tensor("pred", [BL, C, H, W], bf16, kind=ikind).ap()
    # target padded with an edge-duplicated row above and below each image
    targ_d = nc.dram_tensor("target", [BL, H + 2, W], bf16, kind=ikind).ap()
    g_d = nc.dram_tensor("gsums", [BL, 128, GW], bf16, kind="ExternalOutput").ap()
    n_d = nc.dram_tensor("nsums", [128, 5 * BL], f32, kind="ExternalOutput").ap()

    with tile.TileContext(nc) as tc:
        with (
            tc.tile_pool(name="px", bufs=2) as px,
            tc.tile_pool(name="pt", bufs=1) as pt,
            tc.tile_pool(name="ps", bufs=1) as ps,
            tc.tile_pool(name="pr", bufs=2) as pr,
            tc.tile_pool(name="pm", bufs=2) as pm,
            tc.tile_pool(name="pg", bufs=2) as pg,
            tc.tile_pool(name="pn", bufs=1) as pn,
            tc.tile_pool(name="pps", bufs=1, space=bass.MemorySpace.PSUM) as pps,
        ):
            # one psum tile per gram region so a region's matmuls never
            # wait on another region's evacuation
            psA = pps.tile([128, 512], f32, tag="psA", name="psA")
            psB = pps.tile([128, 512], f32, tag="psB", name="psB")
            psD = pps.tile([128, 512], f32, tag="psD", name="psD")
            psE = pps.tile([128, 512], f32, tag="psE", name="psE")
            psF = pps.tile([128, 512], f32, tag="psF", name="psF")
            psC = pps.tile([128, 128], f32, tag="psC", name="psC")
            ncol = pn.tile([128, 5 * BL], f32, tag="ncol", name="ncol")

            def p1_dma_exp(b):
                """Input DMAs (SP queue) + exp on ACT."""
                xg = px.tile([128, C, S, W], bf16, tag="xg")
                Ecv = xg[:].rearrange("p c s w -> p c (s w)")
                for c in range(C):
                    nc.sync.dma_start(
                        xg[:, c], pred_d[b][c].rearrange("(s p) w -> p s w", p=128)
                    )
                    nc.scalar.activation(Ecv[:, c], Ecv[:, c], AF.Exp)
                t8 = pt.tile([128, S, W], bf16, tag="t")
                nc.sync.dma_start(
                    t8[:], targ_d[b][1 : H + 1].rearrange("(s p) w -> p s w", p=128)
                )
                tu8 = pt.tile([128, S, W], bf16, tag="tu")
                nc.sync.dma_start(
                    tu8[:], targ_d[b][0:H].rearrange("(s p) w -> p s w", p=128)
                )
                td8 = pt.tile([128, S, W], bf16, tag="td")
                nc.sync.dma_start(
                    td8[:], targ_d[b][2 : H + 2].rearrange("(s p) w -> p s w", p=128)
                )
                return {"b": b, "t8": t8, "tu8": tu8, "td8": td8, "Ec": Ecv}

            def p1_early_dve(tl):
                """Masks + boundary map: DVE work with no exp dependency."""
                b, t8 = tl["b"], tl["t8"]
                tf = t8[:].rearrange("p s w -> p (s w)")
                m = pm.tile([128, C, FW], bf16, tag="m")
                for c in range(C):
                    nc.vector.tensor_scalar(
                        m[:, c], tf[:], float(c), 0.0, op0=OP.is_equal,
                        op1=OP.add,
                        accum_out=ncol[:, b * 5 + c : b * 5 + c + 1],
                    )
                vn1 = ps.tile([128, FW], bf16, tag="scrA")
                nc.vector.tensor_tensor(
                    vn1[:].rearrange("p (s w) -> p s w", s=S), t8[:], tl["tu8"][:],
                    op=OP.not_equal,
                )
                vn2 = ps.tile([128, S, W], bf16, tag="scrB")
                nc.vector.tensor_tensor(vn2[:], t8[:], tl["td8"][:], op=OP.not_equal)
                hbuf = ps.tile([128, S, HB], bf16, tag="hbuf")
                nc.vector.memset(hbuf[:, :, 0:1], 0.0)
                nc.vector.memset(hbuf[:, :, W : W + 1], 0.0)
                nc.vector.tensor_tensor(
                    hbuf[:, :, 1:W], t8[:, :, 0 : W - 1], t8[:, :, 1:W],
                    op=OP.not_equal,
                )
                nc.vector.tensor_tensor(
                    vn1[:], vn1[:], vn2[:].rearrange("p s w -> p (s w)"),
                    op=OP.max,
                )
                nc.vector.tensor_tensor(
                    vn2[:], hbuf[:, :, 0:W], hbuf[:, :, 1 : W + 1], op=OP.max,
                )
                nc.vector.tensor_tensor(
                    vn1[:], vn1[:], vn2[:].rearrange("p s w -> p (s w)"),
                    op=OP.max,
                )
                # w = boundary AND (t>0)  ==  c1 > m_0
                w = pr.tile([128, FW], bf16, tag="w")
                nc.vector.tensor_tensor(w[:], vn1[:], m[:, 0], op=OP.is_gt)
                tl["m"] = m
                tl["w"] = w

            def p1_s(tl):
                """s (DVE, needs exp) -> ln/r2 (ACT)."""
                Ec = tl["Ec"]
                scrA = ps.tile([128, FW], bf16, tag="scrA")
                nc.vector.tensor_tensor(scrA[:], Ec[:, 0], Ec[:, 1], op=OP.add)
                scrB = ps.tile([128, FW], bf16, tag="scrB")
                nc.vector.tensor_tensor(
                    scrB[:], Ec[:, 2], Ec[:, 3], op=OP.add)
                nc.vector.tensor_tensor(scrA[:], scrA[:], scrB[:], op=OP.add)
                sden = ps.tile([128, FW], bf16, tag="sden")
                nc.vector.tensor_tensor(sden[:], scrA[:], Ec[:, 4], op=OP.add)
                ln_s = ps.tile([128, FW], f32, tag="lns")
                nc.scalar.activation(ln_s[:], sden[:], AF.Ln)
                r2 = pr.tile([128, FW], bf16, tag="r2")
                nc.scalar.activation(r2[:], ln_s[:], AF.Exp, scale=-1.0, bias=-LN2)
                tl["r2"] = r2

            def p1_em(tl):
                """r2w then masked exponentials (DVE)."""
                r2w = pr.tile([128, FW], bf16, tag="r2w")
                nc.vector.tensor_tensor(r2w[:], tl["r2"][:], tl["w"][:], op=OP.mult)
                tl["r2w"] = r2w
                Ec = tl["Ec"]
                Em = pm.tile([128, C, FW], bf16, tag="Em")
                for c in range(C):
                    nc.vector.tensor_tensor(
                        Em[:, c], Ec[:, c], tl["m"][:, c], op=OP.mult)
                tl["Em"] = Em

            def phase1(b):
                # warmup order: softmax chain first so PE can start ASAP
                tl = p1_dma_exp(b)
                p1_s(tl)
                p1_early_dve(tl)
                p1_em(tl)
                return tl

            def mm_region(pt_, statn, mov, nb):
                for k in range(NCH):
                    kk = slice(128 * k, 128 * (k + 1))
                    nc.tensor.matmul(
                        pt_[:, 0 : nb * 128],
                        statn[:, kk],
                        mov[..., kk],
                        start=(k == 0), stop=(k == NCH - 1),
                        skip_group_check=True,
                    )

            def evac(gsb, pt_, off, nb):
                nc.scalar.activation(
                    gsb[:, off : off + nb * 128], pt_[:, 0 : nb * 128], AF.Copy,
                )

            def phase2a(tl, gsb):
                """Gram regions not needing Em, in dependency-readiness
                order: F (w,m from early-DVE), A (r2), D (r2w)."""
                mm_region(psF, tl["w"], tl["m"][:, 1:5], 4)
                evac(gsb, psF, 2048, 4)
                mm_region(psA, tl["r2"], tl["Ec"][:, 0:4], 4)
                evac(gsb, psA, 0, 4)
                mm_region(psD, tl["r2w"], tl["Ec"][:, 1:5], 4)
                evac(gsb, psD, 1024, 4)

            def phase2b(tl, gsb):
                """Em gram regions: PB, PE2, PC (+ evacs) and result DMA."""
                mm_region(psB, tl["r2"], tl["Em"][:, 0:4], 4)
                evac(gsb, psB, 512, 4)
                mm_region(psE, tl["r2w"], tl["Em"][:, 1:5], 4)
                evac(gsb, psE, 1536, 4)
                mm_region(psC, tl["r2"], tl["Em"][:, 4:5], 1)
                evac(gsb, psC, 2560, 1)
                nc.scalar.dma_start(g_d[tl["b"]], gsb[:])

            # Software-pipelined schedule. Per slot b (steady state):
            #   PE: regions without Em for b, then Em regions for b
            #   ACT: evacs(b) interleave; exp(b+1) issued between the two
            #        PE groups so it runs during PE(b)'s Em regions
            #   DVE: masks/wmap(b+1) during exp(b+1), then s/Em/r2w(b+1)
            loop_cm = tc.For_i(0, repeat) if repeat > 1 else contextlib.nullcontext()
            with loop_cm:
                # two-deep pipeline: image b+1's DMAs+exp are issued a full
                # slot ahead (slot b-1), so its s/ln/r2 chain completes
                # before PE switches images.
                tls = {0: phase1(0)}
                if BL > 1:
                    tls[1] = p1_dma_exp(1)
                for b in range(BL):
                    gsb = pg.tile([128, GW], bf16, tag="gsb")
                    phase2a(tls[b], gsb)
                    if b + 1 < BL:
                        p1_early_dve(tls[b + 1])
                        p1_s(tls[b + 1])
                    phase2b(tls[b], gsb)
                    if b + 1 < BL:
                        p1_em(tls[b + 1])
                    if b + 2 < BL:
                        tls[b + 2] = p1_dma_exp(b + 2)
                    tls.pop(b)

            nc.sync.dma_start(n_d[:], ncol[:])

    nc.compile()
    return nc


def _get_nc(BL, C, H, W, repeat=1, **kw):
    key = (BL, C, H, W, repeat, tuple(sorted(kw.items())))
    if key not in _CACHE:
        _CACHE[key] = _build(BL, C, H, W, repeat, **kw)
    return _CACHE[key]


def _finalize(results, BL, C, npix=512 * 512):
    dice_std_all = []
    dice_b_all = []
    for res in results:
        g = np.asarray(res["gsums"]).astype(np.float64)   # [BL, 128, GW]
        ns = np.asarray(res["nsums"]).astype(np.float64)  # [128, 5*BL]
        q = np.arange(128)
        for b in range(BL):
            blocks = g[b].reshape(128, GW // 128, 128)    # [q, block, col]
            d = blocks[q, :, q]                           # [128, 21]
            bs = d.sum(axis=0)                            # 21 block sums
            S1h = bs[0:4]
            S2h = np.concatenate([bs[4:8], bs[20:21]])
            S4h = bs[8:12]
            S5h = bs[12:16]
            NWh = bs[16:20]
            N = ns[:, b * 5 : (b + 1) * 5].sum(axis=0)

            S1 = np.zeros(5)
            S1[:4] = 2.0 * S1h
            S1[4] = npix - S1[:4].sum()
            S2 = 2.0 * S2h
            NW = np.zeros(5)
            NW[1:] = NWh
            wtot = NWh.sum()
            S4 = np.zeros(5)
            S4[1:] = 2.0 * S4h
            S4[0] = wtot - S4[1:].sum()
            S5 = np.zeros(5)
            S5[1:] = 2.0 * S5h

            dice_std_all.append((2.0 * S2 + EPS) / (S1 + N + EPS))
            dice_b_all.append((2.0 * S5 + EPS) / (S4 + NW + EPS))
    loss_std = 1.0 - np.stack(dice_std_all).mean()
    loss_b = 1.0 - np.stack(dice_b_all).mean()
    return np.float32(
        (1.0 - BOUNDARY_WEIGHT) * loss_std + BOUNDARY_WEIGHT * loss_b
    )


def kernel(pred, target):
    import ml_dtypes
    from concourse.bass_utils import run_bass_kernel_spmd

    pred = np.ascontiguousarray(
        np.asarray(pred, dtype=np.float32).astype(ml_dtypes.bfloat16)
    )
    t = np.asarray(target).astype(ml_dtypes.bfloat16)
    target = np.ascontiguousarray(
        np.concatenate([t[:, 0:1], t, t[:, -1:]], axis=1)
    )
    B, C, H, W = pred.shape
    assert B % N_CORES == 0
    BL = B // N_CORES

    nc = _get_nc(BL, C, H, W)
    in_maps = [
        {
            "pred": pred[i * BL : (i + 1) * BL],
            "target": target[i * BL : (i + 1) * BL],
        }
        for i in range(N_CORES)
    ]
    res = run_bass_kernel_spmd(nc, in_maps, list(range(N_CORES)))
    return _finalize([res.results[i] for i in range(N_CORES)], BL, C)
